# revision 6
# baseline (speedup 1.0000x reference)
"""Trainium2 Bass kernel: batched affine bilinear sampling (spatial transformer).

Full inputs: images [32, 512, 512, 3] f32, theta [32, 2, 3] f32.
Data parallel over batch: 8 NeuronCores x 4 images each; ONE SPMD launch per
core processes all four images back-to-back so the GpSimd (SWDGE) gather
stream of image k overlaps the staging DMAs of image k+1.

Device algorithm per image:
  1. Stage the image into a DRAM "quad image"
     imgQ[y*512+x] = [img[y,x,:], img[y,x+1,:], img[y+1,x,:], img[y+1,x+1,:]]
     so the 4 bilinear neighbors of any sample sit in one 48B row.
  2. Compute per-pixel sample coordinates / lerp weights on DVE+ACT from theta
     (exact floor via 2^23 magic-round + compare; exact clamp semantics of the
     reference, including its zero-weight right/bottom overflow edges).
  3. Gather one 48B quad per output pixel via indirect DMA, 128 offsets (one
     per partition) per instruction - the only indirect-DMA configuration that
     is correct on this hardware stack (multi-offset lowering corrupts every
     32nd descriptor; DRAM-dest indirection wedges the device).
  4. Blend and store per 128-column quarter so DVE work and output DMA overlap
     the remaining gather stream via sub-tile dependencies.
"""

import sys
from contextlib import ExitStack

for _p in ("/opt/trn_rl_repo",):
    if _p not in sys.path:
        sys.path.append(_p)

import numpy as np

import concourse.bacc as bacc
import concourse.bass as bass
import concourse.tile as tile
from concourse import mybir
from concourse.bass import IndirectOffsetOnAxis
from concourse.bass_utils import run_bass_kernel_spmd

F32 = mybir.dt.float32
I32 = mybir.dt.int32
OP = mybir.AluOpType
ACTF = mybir.ActivationFunctionType

H = W = 512
P = 128
NBLK = H // P
MAGIC = float(2 ** 23)
N_CORES = 8
IMGS_PER_CORE = 4


def _stage_image(nc, stage_pool, pairs_pool, imgs_flat, imgQ, k):
    """Build the quad image for image k in DRAM."""
    for blk in range(NBLK):
        r0 = blk * P
        loadAB = stage_pool.tile([P, 2, (W + 1) * 3], F32)
        src = bass.AP(
            tensor=imgs_flat.tensor,
            offset=imgs_flat.offset + (k * (H + 2) * W + r0 * W) * 3,
            ap=[[W * 3, P], [W * 3, 2], [1, (W + 1) * 3]],
        )
        nc.scalar.dma_start(out=loadAB, in_=src)
        lab = loadAB.rearrange("p j (w c) -> p j w c", c=3)
        pairs = pairs_pool.tile([P, W, 4, 3], F32)
        nc.scalar.activation(out=pairs[:, :, 0, :], in_=lab[:, 0, 0:W, :], func=ACTF.Copy)
        nc.vector.tensor_copy(out=pairs[:, :, 1, :], in_=lab[:, 0, 1:W + 1, :])
        nc.scalar.activation(out=pairs[:, :, 2, :], in_=lab[:, 1, 0:W, :], func=ACTF.Copy)
        nc.vector.tensor_copy(out=pairs[:, :, 3, :], in_=lab[:, 1, 1:W + 1, :])
        nc.sync.dma_start(
            out=imgQ[r0 * W:(r0 + P) * W, :].rearrange("(p n) c -> p (n c)", p=P),
            in_=pairs.rearrange("p w j c -> p (w j c)"))


def _body(ctx: ExitStack, tc: "tile.TileContext", imgs: bass.AP,
          theta: bass.AP, out: bass.AP):
    nc = tc.nc

    const_pool = ctx.enter_context(tc.tile_pool(name="const", bufs=1))
    th_pool = ctx.enter_context(tc.tile_pool(name="thp", bufs=2))
    stage_pool = ctx.enter_context(tc.tile_pool(name="stage", bufs=2))
    pairs_pool = ctx.enter_context(tc.tile_pool(name="pairs", bufs=2))
    tiny_pool = ctx.enter_context(tc.tile_pool(name="tiny", bufs=2))
    coord_pool = ctx.enter_context(tc.tile_pool(name="coord", bufs=2))
    quad_pool = ctx.enter_context(tc.tile_pool(name="quad", bufs=2))
    blend_pool = ctx.enter_context(tc.tile_pool(name="blend", bufs=2))
    dram_pools = [
        ctx.enter_context(tc.tile_pool(name="drampA", bufs=1, space="DRAM")),
        ctx.enter_context(tc.tile_pool(name="drampB", bufs=1, space="DRAM")),
    ]

    iota_row_i = const_pool.tile([P, W], I32)
    nc.gpsimd.iota(iota_row_i, [[1, W]], base=0, channel_multiplier=0)
    gx = const_pool.tile([P, W], F32)
    nc.vector.tensor_copy(out=gx, in_=iota_row_i)
    nc.vector.tensor_scalar(out=gx, in0=gx, scalar1=2.0 / 511.0, scalar2=-1.0,
                            op0=OP.mult, op1=OP.add)

    iota_col_i = const_pool.tile([P, 1], I32)
    nc.gpsimd.iota(iota_col_i, [[0, 1]], base=0, channel_multiplier=1)
    iotacf = const_pool.tile([P, 1], F32)
    nc.vector.tensor_copy(out=iotacf, in_=iota_col_i)

    zero12 = const_pool.tile([1, 12], F32)
    nc.vector.memset(zero12, 0.0)

    imgs_flat = imgs.rearrange("k h w c -> k (h w c)")

    bbv = [float(np.float32(128.0 * q * (512.0 / 511.0) - 256.0))
           for q in range(NBLK)]

    for k in range(IMGS_PER_CORE):
        imgQ = dram_pools[k % 2].tile([H * W + 1, 12], F32, name=f"imgQ{k}")
        nc.sync.dma_start(out=imgQ[H * W:H * W + 1, :], in_=zero12)
        _stage_image(nc, stage_pool, pairs_pool, imgs_flat, imgQ, k)

        th = th_pool.tile([P, 6], F32, name="th")
        nc.sync.dma_start(
            out=th, in_=theta[k].rearrange("i j -> (i j)").unsqueeze(0)
            .to_broadcast([P, 6]))
        a_ = th[:, 0:1]; b_ = th[:, 1:2]; c_ = th[:, 2:3]
        d_ = th[:, 3:4]; e_ = th[:, 4:5]; f_ = th[:, 5:6]

        A256 = tiny_pool.tile([P, 1], F32, name="A256")
        nc.vector.tensor_scalar_mul(A256, a_, 256.0)
        D256 = tiny_pool.tile([P, 1], F32, name="D256")
        nc.vector.tensor_scalar_mul(D256, d_, 256.0)
        c1x = tiny_pool.tile([P, 1], F32, name="c1x")
        nc.vector.tensor_scalar(out=c1x, in0=c_, scalar1=1.0, scalar2=256.0,
                                op0=OP.add, op1=OP.mult)
        c1y = tiny_pool.tile([P, 1], F32, name="c1y")
        nc.vector.tensor_scalar(out=c1y, in0=f_, scalar1=1.0, scalar2=256.0,
                                op0=OP.add, op1=OP.mult)
        xa = tiny_pool.tile([P, W], F32, name="xa")
        nc.vector.tensor_scalar(out=xa, in0=gx, scalar1=A256, scalar2=None, op0=OP.mult)
        ya = tiny_pool.tile([P, W], F32, name="ya")
        nc.vector.tensor_scalar(out=ya, in0=gx, scalar1=D256, scalar2=None, op0=OP.mult)

        for q in range(NBLK):
            gyb = tiny_pool.tile([P, 1], F32, name="gyb")
            nc.vector.tensor_scalar(out=gyb, in0=iotacf, scalar1=512.0 / 511.0,
                                    scalar2=bbv[q], op0=OP.mult, op1=OP.add)
            sx = tiny_pool.tile([P, 1], F32, name="sx")
            nc.vector.tensor_scalar(out=sx, in0=gyb, scalar1=b_, scalar2=c1x,
                                    op0=OP.mult, op1=OP.add)
            sy = tiny_pool.tile([P, 1], F32, name="sy")
            nc.vector.tensor_scalar(out=sy, in0=gyb, scalar1=e_, scalar2=c1y,
                                    op0=OP.mult, op1=OP.add)

            def coord_side(arow, scol, tag):
                v = coord_pool.tile([P, W], F32, name=f"v{tag}")
                nc.vector.tensor_scalar(out=v, in0=arow, scalar1=scol, scalar2=None,
                                        op0=OP.add)
                r = coord_pool.tile([P, W], F32, name=f"r{tag}")
                nc.scalar.activation(out=r, in_=v, func=ACTF.Copy, bias=MAGIC)
                nc.scalar.activation(out=r, in_=r, func=ACTF.Copy, bias=-MAGIC)
                g = coord_pool.tile([P, W], F32, name=f"g{tag}")
                nc.vector.tensor_tensor(out=g, in0=r, in1=v, op=OP.is_gt)
                nc.vector.tensor_sub(r, r, g)
                nc.vector.tensor_scalar(out=r, in0=r, scalar1=0.0, scalar2=511.0,
                                        op0=OP.max, op1=OP.min)
                p1 = coord_pool.tile([P, W], F32, name=f"p1{tag}")
                nc.vector.tensor_scalar(out=p1, in0=r, scalar1=1.0, scalar2=511.0,
                                        op0=OP.add, op1=OP.min)
                nc.vector.tensor_scalar(out=v, in0=v, scalar1=0.0, scalar2=511.0,
                                        op0=OP.max, op1=OP.min)
                nc.vector.tensor_sub(p1, p1, v)
                nc.vector.tensor_sub(v, v, r)
                return p1, v, r

            u0, u1, x0f = coord_side(xa, sx, "x")
            v0, v1, y0f = coord_side(ya, sy, "y")

            idxf = coord_pool.tile([P, W], F32)
            nc.vector.tensor_scalar(out=idxf, in0=y0f, scalar1=512.0, scalar2=None,
                                    op0=OP.mult)
            nc.vector.tensor_add(idxf, idxf, x0f)
            idxi = coord_pool.tile([P, W], I32)
            nc.vector.tensor_copy(out=idxi, in_=idxf)

            quad = quad_pool.tile([P, W, 12], F32, name="quad")
            QW = W // 4
            for s in range(4):
                for ox in range(s * QW, (s + 1) * QW):
                    nc.gpsimd.indirect_dma_start(
                        out=quad[:, ox, :],
                        out_offset=None,
                        in_=imgQ[:, :],
                        in_offset=IndirectOffsetOnAxis(ap=idxi[:, ox:ox + 1], axis=0),
                    )
                sl = slice(s * QW, (s + 1) * QW)
                q4 = quad[:, sl, :].rearrange("p w (jk c) -> p w jk c", c=3)
                tmp6 = blend_pool.tile([P, QW, 2, 3], F32, name="tmp6")
                u0b = u0[:, sl].unsqueeze(2).unsqueeze(3).to_broadcast([P, QW, 2, 3])
                u1b = u1[:, sl].unsqueeze(2).unsqueeze(3).to_broadcast([P, QW, 2, 3])
                nc.vector.tensor_mul(tmp6, q4[:, :, 1:4:2, :], u1b)
                nc.vector.tensor_mul(q4[:, :, 0:4:2, :], q4[:, :, 0:4:2, :], u0b)
                nc.vector.tensor_add(q4[:, :, 0:4:2, :], q4[:, :, 0:4:2, :], tmp6)
                v0b = v0[:, sl].unsqueeze(2).to_broadcast([P, QW, 3])
                v1b = v1[:, sl].unsqueeze(2).to_broadcast([P, QW, 3])
                outt = blend_pool.tile([P, QW, 3], F32, name="outt")
                tmp3 = blend_pool.tile([P, QW, 3], F32, name="tmp3")
                nc.vector.tensor_mul(outt, q4[:, :, 0, :], v0b)
                nc.vector.tensor_mul(tmp3, q4[:, :, 2, :], v1b)
                nc.vector.tensor_add(outt, outt, tmp3)
                nc.sync.dma_start(out=out[k, q, :, sl, :], in_=outt)


def build_kernel(num_devices: int = N_CORES):
    nc = bacc.Bacc("TRN2", target_bir_lowering=False, debug=False,
                   num_devices=num_devices)
    imgs = nc.dram_tensor("imgs", [IMGS_PER_CORE, H + 2, W, 3], F32,
                          kind="ExternalInput")
    theta = nc.dram_tensor("theta", [IMGS_PER_CORE, 2, 3], F32,
                           kind="ExternalInput")
    out = nc.dram_tensor("out", [IMGS_PER_CORE, NBLK, P, W, 3], F32,
                         kind="ExternalOutput")
    with tile.TileContext(nc) as tc:
        with ExitStack() as ctx:
            _body(ctx, tc, imgs.ap(), theta.ap(), out.ap())
    nc.compile()
    return nc


_NC_CACHE = {}


def run_kernel_spmd(images: np.ndarray, theta: np.ndarray, trace: bool = False):
    B = images.shape[0]
    per = B // N_CORES
    assert per == IMGS_PER_CORE
    if "k4" not in _NC_CACHE:
        _NC_CACHE["k4"] = build_kernel(N_CORES)
    nc = _NC_CACHE["k4"]

    in_maps = []
    for c in range(N_CORES):
        s = np.zeros((per, H + 2, W, 3), np.float32)
        s[:, :H] = images[c * per:(c + 1) * per]
        in_maps.append({
            "imgs": s,
            "theta": np.ascontiguousarray(theta[c * per:(c + 1) * per])
            .astype(np.float32),
        })

    res = run_bass_kernel_spmd(nc, in_maps, core_ids=list(range(N_CORES)),
                               trace=trace)
    out = np.zeros((B, H, W, 3), np.float32)
    for c in range(N_CORES):
        out[c * per:(c + 1) * per] = \
            res.results[c]["out"].reshape(per, H, W, 3)
    return out, res


def kernel(images: np.ndarray, theta: np.ndarray) -> np.ndarray:
    images = np.ascontiguousarray(np.asarray(images), dtype=np.float32)
    theta = np.asarray(theta).astype(np.float32)
    out, _ = run_kernel_spmd(images, theta, trace=False)
    return out


# revision 7
# speedup vs baseline: 2.1343x; 2.1343x over previous
"""Grouped-gather bilinear kernel: one indirect-DMA descriptor serves G
adjacent output pixels (G=4 for |slope|<=1 images, G=2 for |slope|<=3) via a
6x6-pixel block image on an even anchor grid.

Sections per core (one SPMD launch): [Q4, Q4, Q4, Q2].
Vertical grouping via host-side transpose (swapped theta).
"""

import sys
from contextlib import ExitStack

for _p in ("/opt/trn_rl_repo",):
    if _p not in sys.path:
        sys.path.append(_p)

import numpy as np

import concourse.bacc as bacc
import concourse.bass as bass
import concourse.tile as tile
from concourse import mybir
from concourse.bass import IndirectOffsetOnAxis
from concourse.bass_utils import run_bass_kernel_spmd

F32 = mybir.dt.float32
I32 = mybir.dt.int32
OP = mybir.AluOpType
ACTF = mybir.ActivationFunctionType

H = W = 512
P = 128
NBLK = H // P
MAGIC = float(2 ** 23)
N_CORES = 8
PAD = 6
BS = 6           # block side (pixels)
NA = W // 2      # anchors per axis (grid step 2)
EC = BS * BS * 3  # floats per block entry (108)

SECTIONS = ("Q4", "Q4", "Q4", "Q2")
G = {"Q4": 4, "Q2": 2}
CHW = {"Q4": 64, "Q2": 64}  # groups per blend chunk


def _floor_clip(nc, pool, v, lim, tag):
    r = pool.tile(v.shape, F32, name=f"fc{tag}")
    nc.scalar.activation(out=r, in_=v, func=ACTF.Copy, bias=MAGIC)
    nc.scalar.activation(out=r, in_=r, func=ACTF.Copy, bias=-MAGIC)
    g = pool.tile(v.shape, F32, name=f"fg{tag}")
    nc.vector.tensor_tensor(out=g, in0=r, in1=v, op=OP.is_gt)
    nc.vector.tensor_sub(r, r, g)
    nc.vector.tensor_scalar(out=r, in0=r, scalar1=0.0, scalar2=float(lim),
                            op0=OP.max, op1=OP.min)
    return r


def _coord_side(nc, pool, arow, scol, tag):
    v = pool.tile(arow.shape, F32, name=f"v{tag}")
    nc.vector.tensor_scalar(out=v, in0=arow, scalar1=scol, scalar2=None,
                            op0=OP.add)
    r = _floor_clip(nc, pool, v, 511.0, tag)
    p1 = pool.tile(arow.shape, F32, name=f"p1{tag}")
    nc.vector.tensor_scalar(out=p1, in0=r, scalar1=1.0, scalar2=511.0,
                            op0=OP.add, op1=OP.min)
    nc.vector.tensor_scalar(out=v, in0=v, scalar1=0.0, scalar2=511.0,
                            op0=OP.max, op1=OP.min)
    nc.vector.tensor_sub(p1, p1, v)   # u0
    nc.vector.tensor_sub(v, v, r)     # u1
    return p1, v, r


def _stage_image(nc, load_pool, bc_pool, imgs_flat, B, k):
    """B[ay*NA+ax] = img[2*ay+dy, 2*ax+dx, :] for dy,dx in [0,6)."""
    AXC = 32
    nxc = NA // AXC
    for ablk in range(NA // P):
        for xc in range(nxc):
            x0 = 2 * xc * AXC
            cw = 2 * (AXC - 1) + BS
            ld = load_pool.tile([P, BS, cw * 3], F32, name="ld")
            src = bass.AP(
                tensor=imgs_flat.tensor,
                offset=imgs_flat.offset
                + (k * (H + PAD) * W + ablk * 2 * P * W + x0) * 3,
                ap=[[2 * W * 3, P], [W * 3, BS], [1, cw * 3]],
            )
            nc.scalar.dma_start(out=ld, in_=src)
            ldv = ld.rearrange("p r (w c) -> p r w c", c=3)
            bc = bc_pool.tile([P, AXC, BS * BS, 3], F32, name="bc")
            for dy in range(BS):
                for dx in range(BS):
                    srcv = ldv[:, dy, dx:dx + 2 * (AXC - 1) + 1:2, :]
                    if (dy * BS + dx) % 2 == 0:
                        nc.scalar.activation(out=bc[:, :, dy * BS + dx, :],
                                             in_=srcv, func=ACTF.Copy)
                    else:
                        nc.vector.tensor_copy(out=bc[:, :, dy * BS + dx, :],
                                              in_=srcv)
            blk_rows = B[ablk * P * NA:(ablk + 1) * P * NA, :] \
                .rearrange("(p n) c -> p (n c)", p=P)
            nc.sync.dma_start(
                out=blk_rows[:, xc * AXC * EC:(xc * AXC + AXC) * EC],
                in_=bc.rearrange("p a e c -> p (a e c)"))


def _section(nc, pools, consts, imgs_flat, theta, out, k, kind):
    (load_pool, bc_pool, tiny_pool, coord_pool, quad_pool, blend_pool,
     th_pool, dram_pools) = pools
    (gxS, iotacf) = consts
    g_ = G[kind]
    GW = W // g_           # groups per output row
    chw = CHW[kind]
    nch = GW // chw

    B = dram_pools[k % 2].tile([NA * NA, EC], F32, name=f"B{k}")
    _stage_image(nc, load_pool, bc_pool, imgs_flat, B, k)

    th = th_pool.tile([P, 6], F32, name="th")
    nc.sync.dma_start(
        out=th, in_=theta[k].rearrange("i j -> (i j)").unsqueeze(0)
        .to_broadcast([P, 6]))
    a_ = th[:, 0:1]; b_ = th[:, 1:2]; c_ = th[:, 2:3]
    d_ = th[:, 3:4]; e_ = th[:, 4:5]; f_ = th[:, 5:6]

    A256 = tiny_pool.tile([P, 1], F32, name="A256")
    nc.vector.tensor_scalar_mul(A256, a_, 256.0)
    D256 = tiny_pool.tile([P, 1], F32, name="D256")
    nc.vector.tensor_scalar_mul(D256, d_, 256.0)
    c1x = tiny_pool.tile([P, 1], F32, name="c1x")
    nc.vector.tensor_scalar(out=c1x, in0=c_, scalar1=1.0, scalar2=256.0,
                            op0=OP.add, op1=OP.mult)
    c1y = tiny_pool.tile([P, 1], F32, name="c1y")
    nc.vector.tensor_scalar(out=c1y, in0=f_, scalar1=1.0, scalar2=256.0,
                            op0=OP.add, op1=OP.mult)
    gx = gxS[g_]
    xas, yas = [], []
    for sdx in range(g_):
        xa = tiny_pool.tile([P, GW], F32, name=f"xa{sdx}")
        nc.vector.tensor_scalar(out=xa, in0=gx[sdx], scalar1=A256,
                                scalar2=None, op0=OP.mult)
        ya = tiny_pool.tile([P, GW], F32, name=f"ya{sdx}")
        nc.vector.tensor_scalar(out=ya, in0=gx[sdx], scalar1=D256,
                                scalar2=None, op0=OP.mult)
        xas.append(xa)
        yas.append(ya)

    bbv = [float(np.float32(128.0 * q * (512.0 / 511.0) - 256.0))
           for q in range(NBLK)]

    for q in range(NBLK):
        gyb = tiny_pool.tile([P, 1], F32, name="gyb")
        nc.vector.tensor_scalar(out=gyb, in0=iotacf, scalar1=512.0 / 511.0,
                                scalar2=bbv[q], op0=OP.mult, op1=OP.add)
        sx = tiny_pool.tile([P, 1], F32, name="sx")
        nc.vector.tensor_scalar(out=sx, in0=gyb, scalar1=b_, scalar2=c1x,
                                op0=OP.mult, op1=OP.add)
        sy = tiny_pool.tile([P, 1], F32, name="sy")
        nc.vector.tensor_scalar(out=sy, in0=gyb, scalar1=e_, scalar2=c1y,
                                op0=OP.mult, op1=OP.add)

        sides = []
        for sdx in range(g_):
            u0, u1, x0 = _coord_side(nc, coord_pool, xas[sdx], sx, f"x{sdx}")
            v0, v1, y0 = _coord_side(nc, coord_pool, yas[sdx], sy, f"y{sdx}")
            sides.append({"u0": u0, "u1": u1, "x0": x0,
                          "v0": v0, "v1": v1, "y0": y0})

        xmin = coord_pool.tile([P, GW], F32, name="xmin")
        nc.vector.tensor_tensor(out=xmin, in0=sides[0]["x0"],
                                in1=sides[1]["x0"], op=OP.min)
        ymin = coord_pool.tile([P, GW], F32, name="ymin")
        nc.vector.tensor_tensor(out=ymin, in0=sides[0]["y0"],
                                in1=sides[1]["y0"], op=OP.min)
        for sdx in range(2, g_):
            nc.vector.tensor_tensor(out=xmin, in0=xmin,
                                    in1=sides[sdx]["x0"], op=OP.min)
            nc.vector.tensor_tensor(out=ymin, in0=ymin,
                                    in1=sides[sdx]["y0"], op=OP.min)

        hx = coord_pool.tile([P, GW], F32, name="hx")
        nc.vector.tensor_scalar_mul(hx, xmin, 0.5)
        ax = _floor_clip(nc, coord_pool, hx, 255.0, "ax")
        hy = coord_pool.tile([P, GW], F32, name="hy")
        nc.vector.tensor_scalar_mul(hy, ymin, 0.5)
        ay = _floor_clip(nc, coord_pool, hy, 255.0, "ay")
        nc.vector.tensor_scalar_mul(hx, ax, 2.0)   # anchor col in px
        nc.vector.tensor_scalar_mul(hy, ay, 2.0)   # anchor row in px

        for sdx in range(g_):
            s = sides[sdx]
            dx = coord_pool.tile([P, GW], F32, name=f"dx{sdx}")
            nc.vector.tensor_sub(dx, s["x0"], hx)
            dy = coord_pool.tile([P, GW], F32, name=f"dy{sdx}")
            nc.vector.tensor_sub(dy, s["y0"], hy)
            s["dx"], s["dy"] = dx, dy

        idxf = coord_pool.tile([P, GW], F32, name="idxf")
        nc.vector.tensor_scalar(out=idxf, in0=ay, scalar1=float(NA),
                                scalar2=None, op0=OP.mult)
        nc.vector.tensor_add(idxf, idxf, ax)
        idxi = coord_pool.tile([P, GW], I32, name="idxi")
        nc.vector.tensor_copy(out=idxi, in_=idxf)

        for sc in range(nch):
            quad = quad_pool.tile([P, chw, EC], F32, name="quad")
            for oxp in range(sc * chw, (sc + 1) * chw):
                nc.gpsimd.indirect_dma_start(
                    out=quad[:, oxp - sc * chw, :],
                    out_offset=None,
                    in_=B[:, :],
                    in_offset=IndirectOffsetOnAxis(ap=idxi[:, oxp:oxp + 1], axis=0),
                )
            qv = quad.rearrange("p w (r tc) -> p w r tc", r=BS)  # tc = 18
            sl = slice(sc * chw, (sc + 1) * chw)

            # one reusable tile set per chunk (sides overwrite in turn)
            eqx = [blend_pool.tile([P, chw], F32, name=f"ex{j}")
                   for j in range(BS - 1)]
            eqy = [blend_pool.tile([P, chw], F32, name=f"ey{j}")
                   for j in range(BS - 1)]
            wx = [blend_pool.tile([P, chw], F32, name=f"wx{t}")
                  for t in range(BS)]
            wy = [blend_pool.tile([P, chw], F32, name=f"wy{t}")
                  for t in range(BS)]
            wtmp = blend_pool.tile([P, chw], F32, name="wtmp")
            rr = blend_pool.tile([P, chw, 18], F32, name="rr")
            t18 = blend_pool.tile([P, chw, 18], F32, name="t18")
            pxs = [blend_pool.tile([P, chw, 3], F32, name=f"px{j}")
                   for j in range(g_)]
            t3 = blend_pool.tile([P, chw, 3], F32, name="t3")

            for sdx in range(g_):
                s = sides[sdx]

                def wvec(w0, w1, dd, eqs, ws):
                    for j in range(BS - 1):
                        nc.vector.tensor_scalar(out=eqs[j], in0=dd[:, sl],
                                                scalar1=float(j),
                                                scalar2=None,
                                                op0=OP.is_equal)
                    for t in range(BS):
                        if t == 0:
                            nc.vector.tensor_mul(ws[t], w0[:, sl], eqs[0])
                        elif t < BS - 1:
                            nc.vector.tensor_mul(ws[t], w0[:, sl], eqs[t])
                            nc.vector.tensor_mul(wtmp, w1[:, sl], eqs[t - 1])
                            nc.vector.tensor_add(ws[t], ws[t], wtmp)
                        else:
                            nc.vector.tensor_mul(ws[t], w1[:, sl], eqs[t - 1])

                wvec(s["u0"], s["u1"], s["dx"], eqx, wx)
                wvec(s["v0"], s["v1"], s["dy"], eqy, wy)

                # r-sweep: rr[p, w, t*3+c] = sum_r wy_r * B[r, t, c]
                for r in range(BS):
                    wyb = wy[r].unsqueeze(2).to_broadcast([P, chw, 18])
                    if r == 0:
                        nc.vector.tensor_mul(rr, qv[:, :, 0, :], wyb)
                    else:
                        nc.vector.tensor_mul(t18, qv[:, :, r, :], wyb)
                        nc.vector.tensor_add(rr, rr, t18)
                rv = rr.rearrange("p w (t c) -> p w t c", c=3)
                px = pxs[sdx]
                for t in range(BS):
                    wxb = wx[t].unsqueeze(2).to_broadcast([P, chw, 3])
                    if t == 0:
                        nc.vector.tensor_mul(px, rv[:, :, 0, :], wxb)
                    else:
                        nc.vector.tensor_mul(t3, rv[:, :, t, :], wxb)
                        nc.vector.tensor_add(px, px, t3)
                nc.sync.dma_start(
                    out=out[k, q, :, slice(g_ * sc * chw + sdx,
                                           g_ * (sc + 1) * chw, g_), :],
                    in_=px)


def build_quad_kernel(num_devices: int = N_CORES):
    nc = bacc.Bacc("TRN2", target_bir_lowering=False, debug=False,
                   num_devices=num_devices)
    nimg = len(SECTIONS)
    imgs = nc.dram_tensor("imgs", [nimg, H + PAD, W, 3], F32,
                          kind="ExternalInput")
    theta = nc.dram_tensor("theta", [nimg, 2, 3], F32, kind="ExternalInput")
    out = nc.dram_tensor("out", [nimg, NBLK, P, W, 3], F32,
                         kind="ExternalOutput")
    with tile.TileContext(nc) as tc:
        with ExitStack() as ctx:
            load_pool = ctx.enter_context(tc.tile_pool(name="load", bufs=2))
            bc_pool = ctx.enter_context(tc.tile_pool(name="bc", bufs=2))
            tiny_pool = ctx.enter_context(tc.tile_pool(name="tiny", bufs=2))
            coord_pool = ctx.enter_context(tc.tile_pool(name="coord", bufs=1))
            quad_pool = ctx.enter_context(tc.tile_pool(name="quad", bufs=2))
            blend_pool = ctx.enter_context(tc.tile_pool(name="blend", bufs=2))
            th_pool = ctx.enter_context(tc.tile_pool(name="thp", bufs=2))
            const_pool = ctx.enter_context(tc.tile_pool(name="const", bufs=1))
            dram_pools = [
                ctx.enter_context(tc.tile_pool(name="drA", bufs=1, space="DRAM")),
                ctx.enter_context(tc.tile_pool(name="drB", bufs=1, space="DRAM")),
            ]

            iota_row_i = const_pool.tile([P, W], I32)
            nc.gpsimd.iota(iota_row_i, [[1, W]], base=0, channel_multiplier=0)
            gx_full = const_pool.tile([P, W], F32)
            nc.vector.tensor_copy(out=gx_full, in_=iota_row_i)
            nc.vector.tensor_scalar(out=gx_full, in0=gx_full,
                                    scalar1=2.0 / 511.0, scalar2=-1.0,
                                    op0=OP.mult, op1=OP.add)
            gxS = {}
            for g_ in sorted(set(G[s] for s in SECTIONS)):
                gxS[g_] = [gx_full[:, sdx::g_] for sdx in range(g_)]
            iota_col_i = const_pool.tile([P, 1], I32)
            nc.gpsimd.iota(iota_col_i, [[0, 1]], base=0, channel_multiplier=1)
            iotacf = const_pool.tile([P, 1], F32)
            nc.vector.tensor_copy(out=iotacf, in_=iota_col_i)

            pools = (load_pool, bc_pool, tiny_pool, coord_pool, quad_pool,
                     blend_pool, th_pool, dram_pools)
            consts = (gxS, iotacf)
            imgs_flat = imgs.ap().rearrange("k h w c -> k (h w c)")
            for k, kind in enumerate(SECTIONS):
                _section(nc, pools, consts, imgs_flat, theta.ap(), out.ap(),
                         k, kind)
    nc.compile()
    return nc


IMGS_PER_CORE = 4

def _gen_stage_image(nc, stage_pool, pairs_pool, imgs_flat, imgQ, k):
    """Build the quad image for image k in DRAM."""
    for blk in range(NBLK):
        r0 = blk * P
        loadAB = stage_pool.tile([P, 2, (W + 1) * 3], F32)
        src = bass.AP(
            tensor=imgs_flat.tensor,
            offset=imgs_flat.offset + (k * (H + 2) * W + r0 * W) * 3,
            ap=[[W * 3, P], [W * 3, 2], [1, (W + 1) * 3]],
        )
        nc.scalar.dma_start(out=loadAB, in_=src)
        lab = loadAB.rearrange("p j (w c) -> p j w c", c=3)
        pairs = pairs_pool.tile([P, W, 4, 3], F32)
        nc.scalar.activation(out=pairs[:, :, 0, :], in_=lab[:, 0, 0:W, :], func=ACTF.Copy)
        nc.vector.tensor_copy(out=pairs[:, :, 1, :], in_=lab[:, 0, 1:W + 1, :])
        nc.scalar.activation(out=pairs[:, :, 2, :], in_=lab[:, 1, 0:W, :], func=ACTF.Copy)
        nc.vector.tensor_copy(out=pairs[:, :, 3, :], in_=lab[:, 1, 1:W + 1, :])
        nc.sync.dma_start(
            out=imgQ[r0 * W:(r0 + P) * W, :].rearrange("(p n) c -> p (n c)", p=P),
            in_=pairs.rearrange("p w j c -> p (w j c)"))


def _gen_gen_body(ctx: ExitStack, tc: "tile.TileContext", imgs: bass.AP,
          theta: bass.AP, out: bass.AP):
    nc = tc.nc

    const_pool = ctx.enter_context(tc.tile_pool(name="const", bufs=1))
    th_pool = ctx.enter_context(tc.tile_pool(name="thp", bufs=2))
    stage_pool = ctx.enter_context(tc.tile_pool(name="stage", bufs=2))
    pairs_pool = ctx.enter_context(tc.tile_pool(name="pairs", bufs=2))
    tiny_pool = ctx.enter_context(tc.tile_pool(name="tiny", bufs=2))
    coord_pool = ctx.enter_context(tc.tile_pool(name="coord", bufs=2))
    quad_pool = ctx.enter_context(tc.tile_pool(name="quad", bufs=2))
    blend_pool = ctx.enter_context(tc.tile_pool(name="blend", bufs=2))
    dram_pools = [
        ctx.enter_context(tc.tile_pool(name="drampA", bufs=1, space="DRAM")),
        ctx.enter_context(tc.tile_pool(name="drampB", bufs=1, space="DRAM")),
    ]

    iota_row_i = const_pool.tile([P, W], I32)
    nc.gpsimd.iota(iota_row_i, [[1, W]], base=0, channel_multiplier=0)
    gx = const_pool.tile([P, W], F32)
    nc.vector.tensor_copy(out=gx, in_=iota_row_i)
    nc.vector.tensor_scalar(out=gx, in0=gx, scalar1=2.0 / 511.0, scalar2=-1.0,
                            op0=OP.mult, op1=OP.add)

    iota_col_i = const_pool.tile([P, 1], I32)
    nc.gpsimd.iota(iota_col_i, [[0, 1]], base=0, channel_multiplier=1)
    iotacf = const_pool.tile([P, 1], F32)
    nc.vector.tensor_copy(out=iotacf, in_=iota_col_i)

    zero12 = const_pool.tile([1, 12], F32)
    nc.vector.memset(zero12, 0.0)

    imgs_flat = imgs.rearrange("k h w c -> k (h w c)")

    bbv = [float(np.float32(128.0 * q * (512.0 / 511.0) - 256.0))
           for q in range(NBLK)]

    for k in range(IMGS_PER_CORE):
        imgQ = dram_pools[k % 2].tile([H * W + 1, 12], F32, name=f"imgQ{k}")
        nc.sync.dma_start(out=imgQ[H * W:H * W + 1, :], in_=zero12)
        _gen_stage_image(nc, stage_pool, pairs_pool, imgs_flat, imgQ, k)

        th = th_pool.tile([P, 6], F32, name="th")
        nc.sync.dma_start(
            out=th, in_=theta[k].rearrange("i j -> (i j)").unsqueeze(0)
            .to_broadcast([P, 6]))
        a_ = th[:, 0:1]; b_ = th[:, 1:2]; c_ = th[:, 2:3]
        d_ = th[:, 3:4]; e_ = th[:, 4:5]; f_ = th[:, 5:6]

        A256 = tiny_pool.tile([P, 1], F32, name="A256")
        nc.vector.tensor_scalar_mul(A256, a_, 256.0)
        D256 = tiny_pool.tile([P, 1], F32, name="D256")
        nc.vector.tensor_scalar_mul(D256, d_, 256.0)
        c1x = tiny_pool.tile([P, 1], F32, name="c1x")
        nc.vector.tensor_scalar(out=c1x, in0=c_, scalar1=1.0, scalar2=256.0,
                                op0=OP.add, op1=OP.mult)
        c1y = tiny_pool.tile([P, 1], F32, name="c1y")
        nc.vector.tensor_scalar(out=c1y, in0=f_, scalar1=1.0, scalar2=256.0,
                                op0=OP.add, op1=OP.mult)
        xa = tiny_pool.tile([P, W], F32, name="xa")
        nc.vector.tensor_scalar(out=xa, in0=gx, scalar1=A256, scalar2=None, op0=OP.mult)
        ya = tiny_pool.tile([P, W], F32, name="ya")
        nc.vector.tensor_scalar(out=ya, in0=gx, scalar1=D256, scalar2=None, op0=OP.mult)

        for q in range(NBLK):
            gyb = tiny_pool.tile([P, 1], F32, name="gyb")
            nc.vector.tensor_scalar(out=gyb, in0=iotacf, scalar1=512.0 / 511.0,
                                    scalar2=bbv[q], op0=OP.mult, op1=OP.add)
            sx = tiny_pool.tile([P, 1], F32, name="sx")
            nc.vector.tensor_scalar(out=sx, in0=gyb, scalar1=b_, scalar2=c1x,
                                    op0=OP.mult, op1=OP.add)
            sy = tiny_pool.tile([P, 1], F32, name="sy")
            nc.vector.tensor_scalar(out=sy, in0=gyb, scalar1=e_, scalar2=c1y,
                                    op0=OP.mult, op1=OP.add)

            def coord_side(arow, scol, tag):
                v = coord_pool.tile([P, W], F32, name=f"v{tag}")
                nc.vector.tensor_scalar(out=v, in0=arow, scalar1=scol, scalar2=None,
                                        op0=OP.add)
                r = coord_pool.tile([P, W], F32, name=f"r{tag}")
                nc.scalar.activation(out=r, in_=v, func=ACTF.Copy, bias=MAGIC)
                nc.scalar.activation(out=r, in_=r, func=ACTF.Copy, bias=-MAGIC)
                g = coord_pool.tile([P, W], F32, name=f"g{tag}")
                nc.vector.tensor_tensor(out=g, in0=r, in1=v, op=OP.is_gt)
                nc.vector.tensor_sub(r, r, g)
                nc.vector.tensor_scalar(out=r, in0=r, scalar1=0.0, scalar2=511.0,
                                        op0=OP.max, op1=OP.min)
                p1 = coord_pool.tile([P, W], F32, name=f"p1{tag}")
                nc.vector.tensor_scalar(out=p1, in0=r, scalar1=1.0, scalar2=511.0,
                                        op0=OP.add, op1=OP.min)
                nc.vector.tensor_scalar(out=v, in0=v, scalar1=0.0, scalar2=511.0,
                                        op0=OP.max, op1=OP.min)
                nc.vector.tensor_sub(p1, p1, v)
                nc.vector.tensor_sub(v, v, r)
                return p1, v, r

            u0, u1, x0f = coord_side(xa, sx, "x")
            v0, v1, y0f = coord_side(ya, sy, "y")

            idxf = coord_pool.tile([P, W], F32)
            nc.vector.tensor_scalar(out=idxf, in0=y0f, scalar1=512.0, scalar2=None,
                                    op0=OP.mult)
            nc.vector.tensor_add(idxf, idxf, x0f)
            idxi = coord_pool.tile([P, W], I32)
            nc.vector.tensor_copy(out=idxi, in_=idxf)

            quad = quad_pool.tile([P, W, 12], F32, name="quad")
            QW = W // 4
            for s in range(4):
                for ox in range(s * QW, (s + 1) * QW):
                    nc.gpsimd.indirect_dma_start(
                        out=quad[:, ox, :],
                        out_offset=None,
                        in_=imgQ[:, :],
                        in_offset=IndirectOffsetOnAxis(ap=idxi[:, ox:ox + 1], axis=0),
                    )
                sl = slice(s * QW, (s + 1) * QW)
                q4 = quad[:, sl, :].rearrange("p w (jk c) -> p w jk c", c=3)
                tmp6 = blend_pool.tile([P, QW, 2, 3], F32, name="tmp6")
                u0b = u0[:, sl].unsqueeze(2).unsqueeze(3).to_broadcast([P, QW, 2, 3])
                u1b = u1[:, sl].unsqueeze(2).unsqueeze(3).to_broadcast([P, QW, 2, 3])
                nc.vector.tensor_mul(tmp6, q4[:, :, 1:4:2, :], u1b)
                nc.vector.tensor_mul(q4[:, :, 0:4:2, :], q4[:, :, 0:4:2, :], u0b)
                nc.vector.tensor_add(q4[:, :, 0:4:2, :], q4[:, :, 0:4:2, :], tmp6)
                v0b = v0[:, sl].unsqueeze(2).to_broadcast([P, QW, 3])
                v1b = v1[:, sl].unsqueeze(2).to_broadcast([P, QW, 3])
                outt = blend_pool.tile([P, QW, 3], F32, name="outt")
                tmp3 = blend_pool.tile([P, QW, 3], F32, name="tmp3")
                nc.vector.tensor_mul(outt, q4[:, :, 0, :], v0b)
                nc.vector.tensor_mul(tmp3, q4[:, :, 2, :], v1b)
                nc.vector.tensor_add(outt, outt, tmp3)
                nc.sync.dma_start(out=out[k, q, :, sl, :], in_=outt)


def build_gen_kernel(num_devices: int = N_CORES):
    nc = bacc.Bacc("TRN2", target_bir_lowering=False, debug=False,
                   num_devices=num_devices)
    imgs = nc.dram_tensor("imgs", [IMGS_PER_CORE, H + 2, W, 3], F32,
                          kind="ExternalInput")
    theta = nc.dram_tensor("theta", [IMGS_PER_CORE, 2, 3], F32,
                           kind="ExternalInput")
    out = nc.dram_tensor("out", [IMGS_PER_CORE, NBLK, P, W, 3], F32,
                         kind="ExternalOutput")
    with tile.TileContext(nc) as tc:
        with ExitStack() as ctx:
            _gen_body(ctx, tc, imgs.ap(), theta.ap(), out.ap())
    nc.compile()
    return nc




# ---------------- host orchestration ----------------

_CACHE = {}

EPSQ4 = 0.999 * 511.0 / 512.0
EPSQ2 = 2.998 * 511.0 / 512.0


def classify(th):
    a, b = abs(float(th[0, 0])), abs(float(th[0, 1]))
    d, e = abs(float(th[1, 0])), abs(float(th[1, 1]))
    return {
        "q4n": a <= EPSQ4 and d <= EPSQ4,
        "q4t": e <= EPSQ4 and b <= EPSQ4,
        "q2n": a <= EPSQ2 and d <= EPSQ2,
        "q2t": e <= EPSQ2 and b <= EPSQ2,
    }


def transpose_theta(th):
    return np.array([[th[1, 1], th[1, 0], th[1, 2]],
                     [th[0, 1], th[0, 0], th[0, 2]]], np.float32)


def plan_assignment(theta):
    B = theta.shape[0]
    els = [classify(theta[i]) for i in range(B)]
    q4 = [i for i in range(B) if els[i]["q4n"] or els[i]["q4t"]]
    rest = [i for i in range(B) if i not in q4]
    q2 = [i for i in rest if els[i]["q2n"] or els[i]["q2t"]]
    fallback = [i for i in rest if i not in q2]

    n4 = N_CORES * sum(1 for s in SECTIONS if s == "Q4")
    n2 = N_CORES * sum(1 for s in SECTIONS if s == "Q2")
    q4_assigned = q4[:n4]
    overflow = q4[n4:]
    q2_assigned = q2 + overflow
    if len(q2_assigned) > n2:
        fallback += q2_assigned[n2:]
        q2_assigned = q2_assigned[:n2]

    slots = [[None] * len(SECTIONS) for _ in range(N_CORES)]
    it4 = iter(q4_assigned)
    it2 = iter(q2_assigned)
    for c in range(N_CORES):
        for s, kind in enumerate(SECTIONS):
            src = it4 if kind == "Q4" else it2
            i = next(src, None)
            if i is None:
                slots[c][s] = (None, False)
            else:
                el = els[i]
                tr = not el["q4n"] if kind == "Q4" else not el["q2n"]
                slots[c][s] = (i, tr)
    return slots, fallback


def run_kernel_spmd(images: np.ndarray, theta: np.ndarray, trace: bool = False):
    B = images.shape[0]
    theta = theta.astype(np.float32)
    slots, fallback = plan_assignment(theta)

    if "quad" not in _CACHE:
        _CACHE["quad"] = build_quad_kernel(N_CORES)
    nc = _CACHE["quad"]

    nimg = len(SECTIONS)
    in_maps = []
    for c in range(N_CORES):
        slab = np.zeros((nimg, H + PAD, W, 3), np.float32)
        ths = np.zeros((nimg, 2, 3), np.float32)
        for s in range(nimg):
            i, tr = slots[c][s]
            if i is None:
                continue
            if tr:
                slab[s, :H] = images[i].transpose(1, 0, 2)
                ths[s] = transpose_theta(theta[i])
            else:
                slab[s, :H] = images[i]
                ths[s] = theta[i]
        in_maps.append({"imgs": slab, "theta": ths})

    res = run_bass_kernel_spmd(nc, in_maps, core_ids=list(range(N_CORES)),
                               trace=trace)
    out = np.zeros((B, H, W, 3), np.float32)
    for c in range(N_CORES):
        r = res.results[c]["out"].reshape(nimg, H, W, 3)
        for s in range(nimg):
            i, tr = slots[c][s]
            if i is None:
                continue
            out[i] = r[s].transpose(1, 0, 2) if tr else r[s]

    if fallback:
        if "gen" not in _CACHE:
            _CACHE["gen"] = build_gen_kernel(N_CORES)
        ncg = _CACHE["gen"]
        per = IMGS_PER_CORE
        padded = list(fallback)
        while len(padded) % (N_CORES * per):
            padded.append(fallback[-1])
        fb_in = []
        for c in range(N_CORES):
            s = np.zeros((per, H + 2, W, 3), np.float32)
            t = np.zeros((per, 2, 3), np.float32)
            for j in range(per):
                i = padded[(c * per + j) % len(padded)]
                s[j, :H] = images[i]
                t[j] = theta[i]
            fb_in.append({"imgs": s, "theta": t})
        resg = run_bass_kernel_spmd(ncg, fb_in, core_ids=list(range(N_CORES)))
        for c in range(N_CORES):
            rg = resg.results[c]["out"].reshape(per, H, W, 3)
            for j in range(per):
                pos = c * per + j
                if pos < len(padded):
                    out[padded[pos]] = rg[j]
    return out, res


def kernel(images: np.ndarray, theta: np.ndarray) -> np.ndarray:
    images = np.ascontiguousarray(np.asarray(images), dtype=np.float32)
    theta = np.asarray(theta).astype(np.float32)
    out, _ = run_kernel_spmd(images, theta, trace=False)
    return out
